# revision 1
# baseline (speedup 1.0000x reference)
"""AttnBlock3D (GroupNorm -> 1x1 QKV -> full attention over 4096 voxels -> proj -> residual)
as a Bass/Tile kernel on 8 TRN2 NeuronCores.

Sharding: core i handles (batch = i // 4, query-chunk = i % 4) where each query
chunk is 1024 of the 4096 flattened voxels. Each core computes K/V for its full
batch locally (tiny duplicated work), so no collectives are needed.

Kernel structure per core:
  - GroupNorm is folded into the QKV projections: per-channel scale s = rstd*gamma
    and shift t = beta - mean*s are computed on-device from bn_stats + a tiny
    group-aggregation matmul, then applied to the (transposed) projection weights.
    Biases ride along as a 65th "ones" row of the activations.
  - Scores are computed transposed (S_T[k, q] = k_tile^T q) so that the softmax
    denominator is produced by the PE itself: the AV matmul's stationary operand is
    [v^T | ones], giving unnormalized attention output AND the row-sum in one
    accumulation group. exp() runs on the scalar engine with the 1/sqrt(c) scale
    folded in; no max-subtraction is needed (scores are bounded ~|2|).
  - 1/rowsum via exp(-ln(rs)) on the scalar engine (single natural_log_exp
    activation-table load, enforced by patching the greedy set chooser),
    broadcast across partitions with a K=1 matmul, applied after the proj
    matmul (which runs concurrently on the unnormalized AV accumulator).
  - Mixed precision tuned on hardware: x / folded weights / proj operands are
    float32r (fp32 with 11-bit mantissa; PE streams it at full rate, producers
    round for free, DMA-fed tensors pre-rounded on host); k/q/P/v^T are bf16,
    whose 128-column stationary operands unlock the PE fast-weight-load path
    (measured 111us -> 83us). The residual path stays exact fp32. Measured
    output error vs the fp32 reference: ~4.5e-05 relative.
  - PE warm-up matmuls bridge the HAM clock-gate window during the x DMA; the
    GroupNorm rsqrt runs as a quake-seed Newton iteration on the vector engine
    so the scalar engine only ever loads one activation table.
"""

import numpy as np

C = 64          # channels
N = 4096        # flattened voxels per batch (16^3)
NQ = 1024       # query chunk per core
KT = 128        # keys per S_T tile (partition dim of S_T)
NKT = N // KT   # 32 key tiles
NB = 2          # batch
NCORES = 8
EPS = 1e-5
SCALE = C ** -0.5



def _round_f32r(a):
    """Round fp32 array to fp32r encoding (11-bit mantissa, RNE)."""
    u = np.ascontiguousarray(a, np.float32).view(np.uint32).copy()
    u += 0x7FF + ((u >> 12) & 1)
    u &= np.uint32(0xFFFFF000)
    return u.view(np.float32)


def _build_module(reps=1, taps=False, pt_bufs=6, av_lag=1, ndma=4):
    from contextlib import ExitStack

    import concourse.tile as tile
    from concourse import bacc, mybir

    f32 = mybir.dt.float32
    r32 = mybir.dt.float32r
    bf16 = mybir.dt.bfloat16
    AF = mybir.ActivationFunctionType
    ALU = mybir.AluOpType

    nc = bacc.Bacc()

    # Force Ln/Exp to resolve to the combined natural_log_exp_and_others set:
    # the default greedy chooser alternates natural_log / exp_and_others and
    # pays a ~1.3us table load per transition.
    import types

    import bass_rust as _br
    from concourse.hw_specs import get_activation_tables

    def _insert_act_table_loads(self):
        has_act = any(isinstance(i, mybir.InstActivation)
                      for b in self.main_func.blocks for i in b.instructions)
        if not has_act:
            return
        both = {AF.Ln, AF.Exp, AF.Copy}
        tables = []
        for name, funcs in get_activation_tables(self.m.arch).items():
            if name != "natural_log_exp_and_others" and (both & funcs):
                funcs = funcs - both
            tables.append((name, funcs))
        _br.insert_act_table_loads(self, tables)

    nc.insert_act_table_loads = types.MethodType(_insert_act_table_loads, nc)

    # Inputs ([65, ...] activations arrive with a ones-row already appended by host;
    # fp32r inputs are pre-rounded host-side)
    xb = nc.dram_tensor("xb", [C + 1, N], r32, kind="ExternalInput")
    xq = nc.dram_tensor("xq", [C + 1, NQ], r32, kind="ExternalInput")
    xres = nc.dram_tensor("xres", [C, NQ], f32, kind="ExternalInput")
    wqkv = nc.dram_tensor("wqkv", [C, 3 * C], f32, kind="ExternalInput")  # [in, 3*out] (w.T concat)
    bqkv = nc.dram_tensor("bqkv", [1, 3 * C], f32, kind="ExternalInput")
    wp = nc.dram_tensor("wp", [C, C], r32, kind="ExternalInput")          # proj_w.T (pre-rounded)
    bp = nc.dram_tensor("bp", [C, 1], f32, kind="ExternalInput")          # proj_b column
    gam = nc.dram_tensor("gamma", [C, 1], f32, kind="ExternalInput")
    bet = nc.dram_tensor("beta", [C, 1], f32, kind="ExternalInput")
    gmat = nc.dram_tensor("G", [C, C], f32, kind="ExternalInput")         # 0.25 within group
    out = nc.dram_tensor("out", [C, NQ], f32, kind="ExternalOutput")
    tap_tensors = {}
    if taps:
        for nm, shp in [("t_rstd", [C, 1]), ("t_waug", [C + 1, 3 * C]),
                        ("t_q", [C, NQ]), ("t_k", [C, N]), ("t_rrs", [1, NQ]),
                        ("t_av", [C + 1, NQ]), ("t_pt", [KT, NQ])]:
            tap_tensors[nm] = nc.dram_tensor(nm, shp, f32, kind="ExternalOutput")

    with tile.TileContext(nc) as tc:
        with ExitStack() as ctx:
            const = ctx.enter_context(tc.tile_pool(name="const", bufs=1))
            big = ctx.enter_context(tc.tile_pool(name="big", bufs=1))
            small = ctx.enter_context(tc.tile_pool(name="small", bufs=1))
            ptp = ctx.enter_context(tc.tile_pool(name="ptp", bufs=pt_bufs))
            ps_sm = ctx.enter_context(tc.tile_pool(name="ps_sm", bufs=2, space="PSUM"))
            ps_s = ctx.enter_context(tc.tile_pool(name="ps_s", bufs=2, space="PSUM"))
            ps_av = ctx.enter_context(tc.tile_pool(name="ps_av", bufs=1, space="PSUM"))

            dma = nc.sync

            from contextlib import nullcontext
            loop_cm = (tc.For_i(0, reps, 1, hint_engines=(mybir.EngineType.PE,))
                       if reps > 1 else nullcontext())
            with loop_cm:
                # ---- activations (x first, alternating both DMA rings) --------------
                x_sb = big.tile([C + 1, N], r32)
                NDMA = ndma
                for j in range(NDMA):
                    w = N // NDMA
                    dma.dma_start(out=x_sb[:, j * w:(j + 1) * w], in_=xb[:, j * w:(j + 1) * w])
                xq_sb = big.tile([C + 1, NQ], r32)
                dma.dma_start(out=xq_sb[:], in_=xq[:, :])
                xres_sb = big.tile([C, NQ], f32)
                dma.dma_start(out=xres_sb[:], in_=xres[:, :])
                # ---- constant loads (SWDGE ring; keeps the HWDGE ring free for x) ---
                g_sb = const.tile([C, C], f32)
                dma.dma_start(out=g_sb[:], in_=gmat[:, :])
                w_sb = const.tile([C, 3 * C], f32)
                dma.dma_start(out=w_sb[:], in_=wqkv[:, :])
                bq_sb = const.tile([1, 3 * C], f32)
                dma.dma_start(out=bq_sb[:], in_=bqkv[:, :])
                wp_sb = const.tile([C, C], r32)
                dma.dma_start(out=wp_sb[:], in_=wp[:, :])
                bp_sb = const.tile([C, 1], f32)
                dma.dma_start(out=bp_sb[:], in_=bp[:, :])
                gam_sb = const.tile([C, 1], f32)
                dma.dma_start(out=gam_sb[:], in_=gam[:, :])
                bet_sb = const.tile([C, 1], f32)
                dma.dma_start(out=bet_sb[:], in_=bet[:, :])
                ones_f32 = const.tile([KT, C], f32)
                nc.vector.memset(ones_f32[:], 1.0)
                ones1r = const.tile([1, C], r32)
                nc.vector.tensor_copy(ones1r[:], ones_f32[0:1, :])
                zeros128 = const.tile([KT, 1], f32)
                nc.vector.memset(zeros128[:], 0.0)
                magic_sb = const.tile([C, 1], mybir.dt.int32)
                nc.vector.memset(magic_sb[:], 0x5F3759DF)
                xrb = big.tile([C, NQ], f32)
                nc.vector.tensor_scalar_add(xrb[:], in0=xres_sb[:], scalar1=bp_sb[:])

                # warm the PE clock gate (HAM) with throwaway matmuls while the
                # x DMA is in flight, so QKV/scores run at full clock.
                for _w in range(55):
                    pwarm = ps_sm.tile([C, 64], f32, tag="sm")
                    nc.tensor.matmul(pwarm[:], ones_f32[0:C, :], ones_f32[0:C, :])

                # ---- GroupNorm stats (bn_stats over 512-wide spans, 8 of them) ------
                bnst = small.tile([C, 8, 6], f32)
                for j in range(8):
                    nc.vector.bn_stats(out=bnst[:, j, :],
                                       in_=x_sb[0:C, j * 512:(j + 1) * 512].bitcast(f32))
                mv = small.tile([C, 2], f32)
                nc.vector.bn_aggr(out=mv[:], in_=bnst[:])

                # stats2 = [mean_c, E[x^2]_c]
                stats2 = small.tile([C, 2], f32)
                nc.vector.tensor_copy(stats2[:, 0:1], mv[:, 0:1])
                nc.vector.scalar_tensor_tensor(
                    out=stats2[:, 1:2], in0=mv[:, 0:1], scalar=mv[:, 0:1], in1=mv[:, 1:2],
                    op0=ALU.mult, op1=ALU.add,
                )

                # group aggregation: [mean_g, E[x^2]_g] per channel
                psum_g = ps_sm.tile([C, 2], f32, tag="sm")
                nc.tensor.matmul(psum_g[:], g_sb[:], stats2[:])
                mg = small.tile([C, 2], f32)
                nc.vector.tensor_copy(mg[:], psum_g[:])

                # var+eps = (E[x^2]_g + eps) - mean_g^2
                msq = small.tile([C, 1], f32)
                nc.vector.tensor_mul(msq[:], mg[:, 0:1], mg[:, 0:1])
                var = small.tile([C, 1], f32)
                nc.vector.scalar_tensor_tensor(
                    out=var[:], in0=mg[:, 1:2], scalar=float(EPS), in1=msq[:],
                    op0=ALU.add, op1=ALU.subtract,
                )
                # rstd = rsqrt(var+eps) on the vector engine (quake seed + 3
                # Newton steps) so the scalar engine only ever needs Exp and a
                # single activation-table load.
                i32 = mybir.dt.int32
                vh = small.tile([C, 1], f32)
                nc.vector.tensor_scalar_mul(vh[:], in0=var[:], scalar1=0.5)
                u2 = small.tile([C, 1], i32)
                nc.vector.tensor_scalar(out=u2[:], in0=var[:].bitcast(i32),
                                        scalar1=1, scalar2=None,
                                        op0=ALU.arith_shift_right)
                y_i = small.tile([C, 1], i32)
                nc.vector.scalar_tensor_tensor(
                    out=y_i[:], in0=magic_sb[:], scalar=0, in1=u2[:],
                    op0=ALU.add, op1=ALU.subtract,
                )
                rstd = small.tile([C, 1], f32)
                yy = small.tile([C, 1], f32)
                yv = small.tile([C, 1], f32)
                cur = y_i[:].bitcast(f32)
                for _nr in range(3):
                    nc.vector.tensor_mul(yy[:], cur, cur)
                    nc.vector.tensor_mul(yv[:], yy[:], vh[:])
                    nc.vector.tensor_scalar(out=yv[:], in0=yv[:],
                                            scalar1=-1.0, scalar2=1.5,
                                            op0=ALU.mult, op1=ALU.add)
                    nc.vector.tensor_mul(rstd[:], cur, yv[:])
                    cur = rstd[:]

                # s = rstd*gamma ; t = beta - mean*s
                s_vec = small.tile([C, 1], f32)
                nc.vector.tensor_mul(s_vec[:], rstd[:], gam_sb[:])
                ms = small.tile([C, 1], f32)
                nc.vector.tensor_mul(ms[:], mg[:, 0:1], s_vec[:])
                t_vec = small.tile([C, 1], f32)
                nc.vector.tensor_sub(t_vec[:], bet_sb[:], ms[:])

                # ---- fold GN into QKV weights: waug[0:64] = wT*s ; waug[64] = t@wT + b
                waug = small.tile([C + 1, 3 * C], r32)
                nc.vector.tensor_scalar_mul(waug[0:C, :], in0=w_sb[:], scalar1=s_vec[:])
                psum_br = ps_sm.tile([1, 3 * C], f32, tag="sm")
                nc.tensor.matmul(psum_br[:], t_vec[:], w_sb[:])
                nc.vector.tensor_add(waug[C:C + 1, :], psum_br[:], bq_sb[:])

                # ---- Q, K projections (channels on partitions) ----------------------
                q_sb = big.tile([C, NQ], bf16)
                for h in range(NQ // 512):
                    pq = ps_sm.tile([C, 512], f32, tag="sm")
                    nc.tensor.matmul(pq[:], waug[:, 0:C], xq_sb[:, h * 512:(h + 1) * 512])
                    nc.scalar.copy(q_sb[:, h * 512:(h + 1) * 512], pq[:])
                k_sb = big.tile([C, N], bf16)
                for h in range(N // 512):
                    pk = ps_sm.tile([C, 512], f32, tag="sm")
                    nc.tensor.matmul(pk[:], waug[:, C:2 * C], x_sb[:, h * 512:(h + 1) * 512])
                    nc.vector.tensor_copy(k_sb[:, h * 512:(h + 1) * 512], pk[:])

                # ---- V^T tiles: [128 voxels, 64 ch | ones] --------------------------
                vt_sb = big.tile([KT, NKT, C + 1], bf16)
                nc.vector.tensor_copy(vt_sb[:, :, C:C + 1], ones_f32[:, 0:NKT])
                for i in range(NKT):
                    pv = ps_sm.tile([KT, C], f32, tag="sm")
                    nc.tensor.matmul(pv[:], x_sb[:, i * KT:(i + 1) * KT], waug[:, 2 * C:3 * C])
                    nc.vector.tensor_copy(vt_sb[:, i, 0:C], pv[:])

                # ---- attention: S_T tiles -> exp -> AV accumulation -----------------
                pav = ps_av.tile([C + 1, NQ], f32)

                def emit_av(pt_p, i_p):
                    for h in range(NQ // 512):
                        nc.tensor.matmul(pav[:, h * 512:(h + 1) * 512],
                                         vt_sb[:, i_p, :],
                                         pt_p[:, h * 512:(h + 1) * 512],
                                         start=(i_p == 0), stop=(i_p == NKT - 1))

                # Lag the AV matmuls one tile behind the scores in PE program
                # order: the PE is in-order, so AV(i) directly after scores(i)
                # stalls the PE on exp(i); with the lag, scores(i+1) fills
                # that window and the loop runs at the ACT exp rate.
                pending = []
                for i in range(NKT):
                    ps = ps_s.tile([KT, NQ], f32, tag="s")
                    kt = k_sb[:, i * KT:(i + 1) * KT]
                    for h in range(NQ // 512):
                        nc.tensor.matmul(ps[:, h * 512:(h + 1) * 512], kt,
                                         q_sb[:, h * 512:(h + 1) * 512])
                    if av_lag and len(pending) >= av_lag:
                        emit_av(*pending.pop(0))
                    pt = ptp.tile([KT, NQ], bf16, tag="pt")
                    nc.scalar.activation(out=pt[:], in_=ps[:], func=AF.Exp,
                                         bias=zeros128[:], scale=SCALE)
                    if av_lag:
                        pending.append((pt, i))
                    else:
                        emit_av(pt, i)
                for p in pending:
                    emit_av(*p)

                # ---- tail: proj on unnormalized AV runs concurrently with the
                # 1/rowsum branch (DVE reciprocal approx + f32 broadcast matmul);
                # everything is split in 512-halves so the stages pipeline.
                av_r = big.tile([C, NQ], r32)
                lrs = small.tile([1, NQ], f32)
                rrs = small.tile([1, NQ], r32)
                bc_sb = big.tile([C, NQ], f32)
                t_sb = big.tile([C, NQ], f32)
                out_sb = big.tile([C, NQ], f32)
                pbc = ps_s.tile([KT, NQ], f32, tag="s")
                pout = ps_s.tile([KT, NQ], f32, tag="s")
                for h in range(NQ // 256):
                    hs = slice(h * 256, (h + 1) * 256)
                    nc.scalar.activation(out=lrs[:, hs], in_=pav[C:C + 1, hs],
                                         func=AF.Ln, bias=zeros128[0:1])
                    nc.scalar.activation(out=rrs[:, hs], in_=lrs[:, hs],
                                         func=AF.Exp, bias=zeros128[0:1], scale=-1.0)
                    nc.scalar.copy(av_r[:, hs], pav[0:C, hs])
                    nc.tensor.matmul(pout[0:C, hs], wp_sb[:], av_r[:, hs])
                    nc.tensor.matmul(pbc[0:C, hs], ones1r[0:1, :], rrs[:, hs])
                    nc.scalar.copy(bc_sb[:, hs], pbc[0:C, hs])
                    nc.vector.tensor_mul(t_sb[:, hs], pout[0:C, hs], bc_sb[:, hs])
                    nc.vector.tensor_add(out_sb[:, hs], t_sb[:, hs], xrb[:, hs])
                    dma.dma_start(out=out[:, hs], in_=out_sb[:, hs])
                if taps:
                    dma.dma_start(out=tap_tensors["t_rstd"][:, :], in_=rstd[:])
                    dma.dma_start(out=tap_tensors["t_waug"][:, :], in_=waug[:].bitcast(f32))
                    dma.dma_start(out=tap_tensors["t_q"][:, :], in_=q_sb[:].bitcast(f32))
                    dma.dma_start(out=tap_tensors["t_k"][:, :], in_=k_sb[:].bitcast(f32))
                    dma.dma_start(out=tap_tensors["t_rrs"][:, :], in_=rrs[:])
                    av_f = big.tile([C + 1, NQ], f32)
                    nc.vector.tensor_copy(av_f[:], pav[:])
                    dma.dma_start(out=tap_tensors["t_av"][:, :], in_=av_f[:])
                    dma.dma_start(out=tap_tensors["t_pt"][:, :], in_=pt[:].bitcast(f32))

    return nc


_cache = {}


def _get_module(finalized=True, reps=1):
    nc = _cache.get(reps)
    if nc is None:
        nc = _cache[reps] = _build_module(reps)
    if finalized and not nc.is_finalized():
        nc.finalize()
    return nc


def make_in_maps(x, norm_w, norm_b, q_w, q_b, k_w, k_b, v_w, v_b, proj_w, proj_b):
    f = np.float32
    x = np.asarray(x, f).reshape(NB, C, N)
    xr = _round_f32r(x)
    ones_n = np.ones((1, N), f)
    ones_q = np.ones((1, NQ), f)
    wqkv = np.concatenate([np.asarray(q_w, f).T, np.asarray(k_w, f).T,
                           np.asarray(v_w, f).T], axis=1)
    bqkv = np.concatenate([np.asarray(q_b, f), np.asarray(k_b, f),
                           np.asarray(v_b, f)])[None, :]
    gmat = np.zeros((C, C), f)
    for g in range(16):
        gmat[g * 4:(g + 1) * 4, g * 4:(g + 1) * 4] = 0.25
    in_maps = []
    for core in range(NCORES):
        b, ch = divmod(core, NCORES // NB)
        xb_full = np.concatenate([xr[b], ones_n], axis=0)
        xq_c = np.concatenate([xr[b][:, ch * NQ:(ch + 1) * NQ], ones_q], axis=0)
        in_maps.append({
            "xb": xb_full,
            "xq": np.ascontiguousarray(xq_c),
            "xres": np.ascontiguousarray(x[b][:, ch * NQ:(ch + 1) * NQ]),
            "wqkv": np.ascontiguousarray(wqkv),
            "bqkv": bqkv,
            "wp": _round_f32r(np.asarray(proj_w, f).T),
            "bp": np.asarray(proj_b, f)[:, None],
            "gamma": np.asarray(norm_w, f)[:, None],
            "beta": np.asarray(norm_b, f)[:, None],
            "G": gmat,
        })
    return in_maps


def assemble_output(results):
    outf = np.zeros((NB, C, N), np.float32)
    for core in range(NCORES):
        b, ch = divmod(core, NCORES // NB)
        outf[b][:, ch * NQ:(ch + 1) * NQ] = np.asarray(results[core]["out"])
    return outf.reshape(NB, C, 16, 16, 16)


def kernel(**inputs) -> np.ndarray:
    from concourse.bass_utils import run_bass_kernel_spmd

    nc = _get_module()
    in_maps = make_in_maps(**inputs)
    res = run_bass_kernel_spmd(nc, in_maps, list(range(NCORES)))
    return assemble_output(res.results)

